# revision 44
# baseline (speedup 1.0000x reference)
"""Ragged GQA attention block (QKV proj + RoPE + paged-KV attention + WO proj)
on 8 TRN2 NeuronCores, tensor-parallel over heads.

v4: dense-PE restructure.
 - Host packs every DRAM tensor partition-major so each DMA is 128 partitions
   x contiguous >=2KB lines, issued in ~1MB transfers in consumption order.
 - K and V projections merged chunk-major (4 MMs per arriving x chunk) so the
   PE stays busy while x streams in.
 - q_ps PSUM pool lives in the outer scope (banks 0-1) so the Q0 projection
   issues immediately after the last K/V matmul -- no wait on the K-rope
   reading the K/V PSUM banks (v3 lost 8us there).
 - A tiny warmup AllGather fires at t~0 to absorb the ~11us ncfw first-
   collective delay and align the 8 cores.
 - Rope runs on the vector engine (gpsimd only rings collective doorbells);
   the per-head attnT spill DMA is on sync so exps never queue behind it.
 - Scores for multiple kv blocks are packed into shared [128,512] PSUM banks
   with ONE exp ACTIVATE per pack; small proj slabs of the NEXT head are
   interleaved right after each batch's scores so exp latency never idles the
   PE, while keeping each head's attention front-loaded (early AG triggers).
 - Per-batch PV matmuls emitted before the ones-matmul sums so the stationary
   weight (Vnat vs ones) doesn't swap every matmul.
All matmul inputs bf16, fp32 PSUM accumulation.
"""

import math
import numpy as np

H, KVH, HD = 32, 8, 128
HIDDEN = H * HD            # 4096
T = 1024
TOTAL_KV = 3072
ROPE_THETA = 10000.0
N_CORES = 8
QH_PER = H // N_CORES      # 4 q heads per core
D2 = HD // 2
SCALE = 1.0 / math.sqrt(HD)
NEG = -1.0e30
KCH = HIDDEN // 128        # 32 contraction chunks
NTB = T // 128             # 8 token blocks
NKVB = TOTAL_KV // 128     # 24 kv blocks

from contextlib import ExitStack

import concourse.bacc as bacc
import concourse.mybir as mybir
import concourse.tile as tile
from concourse.masks import make_identity
from concourse.bass_utils import run_bass_kernel_spmd

dt = mybir.dt
BF = dt.bfloat16
F32 = dt.float32
AFT = mybir.ActivationFunctionType


def build_nc(seqstarts, kvstarts, cachestarts, start_pos):
    """Trace + compile the SPMD Bass program, specialized to the offsets."""
    seqstarts = [int(v) for v in seqstarts]
    kvstarts = [int(v) for v in kvstarts]
    cachestarts = [int(v) for v in cachestarts]
    start_pos = [int(v) for v in start_pos]
    NB = len(start_pos)
    assert len(seqstarts) == NB + 1 and len(kvstarts) == NB + 1
    assert seqstarts[-1] == T and kvstarts[-1] == TOTAL_KV
    for v in seqstarts + kvstarts + start_pos:
        assert v % 128 == 0, "offsets must be 128-aligned"
    B = []
    for b in range(NB):
        s0, s1 = seqstarts[b], seqstarts[b + 1]
        kb, sp = kvstarts[b], start_pos[b]
        S = s1 - s0
        L = kvstarts[b + 1] - kb
        assert L == sp + S, "kv stream length must equal prefix + new tokens"
        assert S <= 512, "per-request seqlen > 512 not supported"
        B.append((s0, s1, S, kb, L, sp, cachestarts[b]))
    CACHED = sum(b[5] for b in B)           # total cached kv positions

    nc = bacc.Bacc(
        "TRN2", target_bir_lowering=False, debug=False, num_devices=N_CORES
    )
    # all inputs pre-packed partition-major on host: [128, free...]
    xT_d = nc.dram_tensor("xp", [128, KCH * T], BF, kind="ExternalInput").ap()
    # wqkv col-blocks [K, V, Q0..Q3]: [128, cb, k, c]
    wqkv_d = nc.dram_tensor(
        "wqkv_p", [128, 6 * KCH * 128], BF, kind="ExternalInput"
    ).ap()
    # wo rows packed in kernel consumption order i=(h,r): rows of head 4r+h
    wo_d = nc.dram_tensor("wo_p", [128, H * 512], BF, kind="ExternalInput").ap()
    # cached K (transposed, perm'd) / V (natural blocks), only used slices,
    # packed densely in batch order
    ck_d = nc.dram_tensor(
        "ck_p", [128, max(CACHED, 128)], BF, kind="ExternalInput"
    ).ap()
    cv_d = nc.dram_tensor(
        "cv_p", [128, max(CACHED, 128)], BF, kind="ExternalInput"
    ).ap()
    # consts (bf16, cos/sin at partitions 0-63): cols [0:T) cosq*s,
    # [T:2T) sinq*s, [2T:3T) cosk, [3T:4T) sink, [4T:4T+128) triT
    NCONST = 4 * T + 128
    consts_d = nc.dram_tensor(
        "consts", [128, NCONST], BF, kind="ExternalInput"
    ).ap()
    outT_d = nc.dram_tensor("outT", [512, T], BF, kind="ExternalOutput").ap()

    ag_out = [
        nc.dram_tensor(
            f"ag_out_{h}", [N_CORES * HD, T], BF, addr_space="Shared"
        ).ap()
        for h in range(QH_PER)
    ]
    warm_out = nc.dram_tensor(
        "warm_out", [N_CORES * 128, 16], BF, addr_space="Shared"
    ).ap()

    with tile.TileContext(nc) as tc:
        with ExitStack() as es:
            ec = es.enter_context
            cpool = ec(tc.tile_pool(name="consts", bufs=1))
            xT_pool = ec(tc.tile_pool(name="xT", bufs=1))
            w_pool = ec(tc.tile_pool(name="w", bufs=4))
            kt_pool = ec(tc.tile_pool(name="KT", bufs=1))
            v_pool = ec(tc.tile_pool(name="Vnat", bufs=1))
            qt_pool = ec(tc.tile_pool(name="QT", bufs=1))
            at_pool = ec(tc.tile_pool(name="attnT", bufs=1))
            rope_pool = ec(tc.tile_pool(name="rope", bufs=2))
            ex_pool = ec(tc.tile_pool(name="ex", bufs=4))
            st_pool = ec(tc.tile_pool(name="st", bufs=2))
            vts_pool = ec(tc.tile_pool(name="vts", bufs=1))
            dramb = ec(tc.tile_pool(name="dramb", bufs=5, space="DRAM"))

            ident_bf = cpool.tile([128, 128], BF)
            make_identity(nc, ident_bf[:])
            ones_kv = cpool.tile([128, 1], BF)
            nc.vector.memset(ones_kv[:], 1.0)
            onesb = cpool.tile([1, 128], BF)
            nc.vector.memset(onesb[:], 1.0)
            warm_src = cpool.tile([128, 16], BF)
            nc.vector.memset(warm_src[:], 0.0)
            consts = cpool.tile([128, NCONST], BF)
            cosqT = consts[0:64, 0:T]
            sinqT = consts[0:64, T : 2 * T]
            coskT = consts[0:64, 2 * T : 3 * T]
            sinkT = consts[0:64, 3 * T : 4 * T]
            triT = consts[:, 4 * T : 4 * T + 128]

            xTsb = xT_pool.tile([128, KCH, T], BF)
            KT = kt_pool.tile([128, TOTAL_KV], BF)
            Vnat = v_pool.tile([128, NKVB, HD], BF)
            QT4 = qt_pool.tile([128, QH_PER, T], BF)
            attnT_sb = at_pool.tile([128, QH_PER, T], BF)
            VTsb = vts_pool.tile([128, T], BF)

            # ---- warmup collective: absorb ncfw boot, align cores ---------
            warm_in = dramb.tile([128, 16], BF, name="warm")
            nc.sync.dma_start(warm_in[:], warm_src[:])
            nc.gpsimd.collective_compute(
                "AllGather",
                mybir.AluOpType.bypass,
                replica_groups=[list(range(N_CORES))],
                ins=[warm_in.opt()],
                outs=[warm_out[:]],
            )

            # ---- input DMAs, partition-contiguous, in consumption order ---
            # single-tag ring of 4: Q2 reuses K's slot, Q3 reuses V's
            wsb = {}

            def load_w(cb):
                wsb[cb] = w_pool.tile([128, KCH, 128], BF, tag="w",
                                      name=f"w_{cb}")
                nc.sync.dma_start(
                    wsb[cb][:],
                    wqkv_d[:, cb * KCH * 128 : (cb + 1) * KCH * 128].rearrange(
                        "p (k c) -> p k c", c=128
                    ),
                )

            def load_x(k0, k1):
                nc.sync.dma_start(
                    xTsb[:, k0:k1, :],
                    xT_d[:, k0 * T : k1 * T].rearrange("p (k t) -> p k t", t=T),
                )

            load_w(0)                      # K weights
            load_x(0, 1)
            load_x(1, 2)
            load_w(1)                      # V weights
            load_x(2, 6)
            nc.sync.dma_start(consts[:], consts_d[:])
            for g in range(6, KCH, 4):
                load_x(g, min(g + 4, KCH))
            # cached K -> KT columns, cached V -> Vnat blocks (dense packs)
            off = 0
            for (s0, s1, S, kb, L, sp, cs) in B:
                if sp:
                    nc.sync.dma_start(
                        KT[:, kb : kb + sp], ck_d[:, off : off + sp]
                    )
                    nc.sync.dma_start(
                        Vnat[:, kb // 128 : (kb + sp) // 128, :],
                        cv_d[:, off : off + sp].rearrange(
                            "p (blk c) -> p blk c", c=128
                        ),
                    )
                    off += sp
            load_w(2)                      # Q0
            load_w(3)                      # Q1

            # WO weights + pool opened early so the wo DMA issues before the
            # per-head agi DMAs in the sync FIFO
            wos_pool = ec(tc.tile_pool(name="wos", bufs=1))
            wosb = wos_pool.tile([128, H, 512], BF)
            nc.sync.dma_start(
                wosb[:], wo_d.rearrange("p (i c) -> p i c", c=512)
            )
            load_w(4)                      # Q2 (reuses K slot, waits free)
            load_w(5)                      # Q3 (reuses V slot)

            def rope(top, bot, cosT, sinT, c0, c1, otop, obot, tag):
                # muls on vector (read PSUM); sub/add on gpsimd (SBUF-only,
                # otherwise idle) -- halves the vector-engine rope cost
                cosv, sinv = cosT[:, c0:c1], sinT[:, c0:c1]
                n = c1 - c0
                t1 = rope_pool.tile([64, 512], F32, tag="ta", name=f"t1_{tag}")
                t2 = rope_pool.tile([64, 512], F32, tag="tb", name=f"t2_{tag}")
                nc.vector.tensor_mul(t1[:, 0:n], top, cosv)
                nc.vector.tensor_mul(t2[:, 0:n], bot, sinv)
                nc.gpsimd.tensor_sub(otop, t1[:, 0:n], t2[:, 0:n])
                t3 = rope_pool.tile([64, 512], F32, tag="ta", name=f"t3_{tag}")
                t4 = rope_pool.tile([64, 512], F32, tag="tb", name=f"t4_{tag}")
                nc.vector.tensor_mul(t3[:, 0:n], top, sinv)
                nc.vector.tensor_mul(t4[:, 0:n], bot, cosv)
                nc.gpsimd.tensor_add(obot, t3[:, 0:n], t4[:, 0:n])

            def proj_chunks(h, half, pQ, k0, k1):
                c0 = half * 512
                for k in range(k0, k1):
                    nc.tensor.matmul(
                        pQ[:], wsb[2 + h][:, k, :],
                        xTsb[:, k, c0 : c0 + 512],
                        start=(k == 0), stop=(k == KCH - 1),
                    )

            def rope_q(h, half, pQ):
                c0 = half * 512
                rope(pQ[0:64, :], pQ[64:128, :], cosqT, sinqT, c0, c0 + 512,
                     QT4[0:64, h, c0 : c0 + 512],
                     QT4[64:128, h, c0 : c0 + 512], tag=f"q{h}_{half}")

            # q proj PSUM opened before kv_ps (gets banks 0-1, so Q0 doesn't
            # wait on K/V PSUM evacuation); closed explicitly before WO
            esq = ExitStack()
            q_ps = esq.enter_context(tc.tile_pool(name="qps", bufs=2,
                                                  space="PSUM"))

            # ---- K+V projection (merged chunk-major) + Q0, V transpose ----
            with ExitStack() as es1:
                kv_ps = es1.enter_context(
                    tc.tile_pool(name="kvps", bufs=1, space="PSUM")
                )
                vt_ps = es1.enter_context(
                    tc.tile_pool(name="vtps", bufs=1, space="PSUM")
                )
                pK = kv_ps.tile([128, T], F32, tag="pk")
                pV = kv_ps.tile([128, T], F32, tag="pv")

                # K + V merged chunk-major: 4 MMs per arriving x chunk keeps
                # the PE dense through the x stream
                for k in range(KCH):
                    st, sp_ = (k == 0), (k == KCH - 1)
                    for cb, ps in ((0, pK), (1, pV)):
                        for half in range(2):
                            nc.tensor.matmul(
                                ps[:, half * 512 : (half + 1) * 512],
                                wsb[cb][:, k, :],
                                xTsb[:, k, half * 512 : (half + 1) * 512],
                                start=st, stop=sp_,
                            )
                nc.scalar.copy(VTsb[:], pV[:])

                # vector FIFO: K-rope of batch 0 first (attn0's b0 needs it)
                (s0, s1, S, kb, L, sp, cs) = B[0]
                rope(pK[0:64, s0:s1], pK[64:128, s0:s1], coskT, sinkT, s0, s1,
                     KT[0:64, kb + sp : kb + sp + S],
                     KT[64:128, kb + sp : kb + sp + S], tag="k0")

                # Q0 half0 issues right after the K/V matmuls -- q_ps banks
                # are its own, so no wait on pK/pV evacuation
                pq0 = {}
                pq0[0] = q_ps.tile([128, 512], F32, tag="pq", name="pq_0_0")
                proj_chunks(0, 0, pq0[0], 0, KCH)

                vtp = vt_ps.tile([128, NTB, 128], BF, tag="vt")
                for tb in range(NTB):
                    nc.tensor.transpose(
                        vtp[:, tb, :], VTsb[:, tb * 128 : (tb + 1) * 128],
                        ident_bf[:],
                    )
                rope_q(0, 0, pq0[0])
                for (s0, s1, S, kb, L, sp, cs) in B[1:]:
                    d = kb + sp
                    rope(pK[0:64, s0:s1], pK[64:128, s0:s1],
                         coskT, sinkT, s0, s1,
                         KT[0:64, d : d + S], KT[64:128, d : d + S],
                         tag=f"k{s0}")

                pq0[1] = q_ps.tile([128, 512], F32, tag="pq", name="pq_0_1")
                proj_chunks(0, 1, pq0[1], 0, KCH)
                for (s0, s1, S, kb, L, sp, cs) in B:
                    tb0 = s0 // 128
                    nb = S // 128
                    blk0 = (kb + sp) // 128
                    nc.scalar.copy(
                        Vnat[:, blk0 : blk0 + nb, :],
                        vtp[:, tb0 : tb0 + nb, :],
                    )
                rope_q(0, 1, pq0[1])

            # ---- per-head attention (+ next-head proj slabs), AllGather ---
            with ExitStack() as es2:
                ec2 = es2.enter_context
                # sc ring of 3 lets three score-packs pipeline through the
                # exp ACTIVATE chain; sums drops to 1 bank to pay for it
                sc_ps = ec2(tc.tile_pool(name="scps", bufs=3, space="PSUM"))
                at_ps = ec2(tc.tile_pool(name="atps", bufs=2, space="PSUM"))
                sum_ps = ec2(tc.tile_pool(name="sumps", bufs=1, space="PSUM"))

                # score-block packs per batch: pack consecutive kv blocks into
                # one [128,512] PSUM bank, ONE exp per pack
                def batch_packs(b):
                    s0, s1, S, kb, L, sp, cs = B[b]
                    packs, cur, width = [], [], 0
                    for j in range(L // 128):
                        dlo = 128 * j - sp
                        c_lo = max(0, dlo)
                        N = S - c_lo
                        if width + N > 512:
                            packs.append(cur)
                            cur, width = [], 0
                        cur.append((j, c_lo, N, width, dlo))
                        width += N
                    if cur:
                        packs.append(cur)
                    return packs

                def emit_scores(h, b):
                    s0, s1, S, kb, L, sp, cs = B[b]
                    out = []
                    for pi, pack in enumerate(batch_packs(b)):
                        sc = sc_ps.tile([128, 512], F32, tag="sc",
                                        name=f"sc_{h}_{b}_{pi}")
                        for (j, c_lo, N, off, dlo) in pack:
                            nc.tensor.matmul(
                                sc[:, off : off + N],
                                KT[:, kb + 128 * j : kb + 128 * j + 128],
                                QT4[:, h, s0 + c_lo : s0 + S],
                                start=True, stop=True,
                            )
                            if dlo >= 0:
                                m = min(128, S - dlo)
                                nc.vector.tensor_add(
                                    sc[:, off : off + m],
                                    sc[:, off : off + m], triT[:, 0:m],
                                )
                        w = pack[-1][3] + pack[-1][2]
                        ex = ex_pool.tile([128, 512], BF, tag="ex",
                                          name=f"ex_{h}_{b}_{pi}")
                        nc.scalar.activation(
                            ex[:, 0:w], sc[:, 0:w], AFT.Exp,
                            bias=0.0, scale=1.0,
                        )
                        for (j, c_lo, N, off, dlo) in pack:
                            out.append((j, c_lo, N, ex, off))
                    return out

                def emit_pv(h, b, tiles, atps, sums):
                    s0, s1, S, kb, L, sp, cs = B[b]
                    base = 0 if s1 <= 512 else 512
                    nj = len(tiles)
                    # all PV matmuls first (stationary = Vnat[j]), then all
                    # sums (stationary = ones loaded ONCE for the whole run)
                    for idx, (j, c_lo, N, ex, off) in enumerate(tiles):
                        nc.tensor.matmul(
                            atps[:, s0 - base + c_lo : s0 - base + S],
                            Vnat[:, kb // 128 + j, :],
                            ex[:, off : off + N],
                            start=(idx == 0), stop=(idx == nj - 1),
                        )
                    for idx, (j, c_lo, N, ex, off) in enumerate(tiles):
                        nc.tensor.matmul(
                            sums[0:1, s0 - base + c_lo : s0 - base + S],
                            ones_kv[:, 0:1],
                            ex[:, off : off + N],
                            start=(idx == 0), stop=(idx == nj - 1),
                        )

                def emit_attn(h, interleave=None):
                    # interleave(b): emit a small slab of the NEXT head's
                    # projection after batch b's scores -- covers exp latency
                    # without pushing this head's completion late
                    state = {}
                    agi = dramb.tile([128, T], BF, name=f"agi{h}")

                    def get_half(half):
                        if half not in state:
                            state[half] = (
                                at_ps.tile([128, 512], F32, tag="at",
                                           name=f"at_{h}_{half}"),
                                sum_ps.tile([1, 512], F32, tag="sums",
                                            name=f"sums_{h}_{half}"),
                            )
                        return state[half]

                    def normalize(half):
                        # attnT[:, c] *= 1/sums[c] via PE broadcast
                        atps, sums = state[half]
                        c0, c1 = half * 512, (half + 1) * 512
                        sumsb = st_pool.tile([1, 512], BF, tag="sumsb",
                                             name=f"sumsb_{h}_{half}")
                        nc.scalar.copy(sumsb[:], sums[:])
                        sB = sc_ps.tile([128, 512], F32, tag="sc",
                                        name=f"sB_{h}_{half}")
                        nc.tensor.matmul(
                            sB[:], onesb[:], sumsb[:],
                            start=True, stop=True,
                        )
                        rinv = st_pool.tile([128, 512], F32, tag="rinv",
                                            name=f"rinv_{h}_{half}")
                        nc.vector.reciprocal_approx_fast(rinv[:], sB[:])
                        nc.vector.tensor_mul(
                            attnT_sb[:, h, c0:c1], atps[:], rinv[:]
                        )
                        # spill this half right away: the AllGather doorbell
                        # then only waits on the second (smaller) DMA
                        nc.sync.dma_start(
                            agi[:, c0:c1], attnT_sb[:, h, c0:c1]
                        )

                    halfA = [b for b in range(NB) if B[b][1] <= 512]
                    assert halfA and all(
                        B[b][0] >= 512 for b in range(NB) if b not in halfA
                    ), "batches must not straddle the 512-token boundary"

                    def pv_for(b, tiles):
                        half = 0 if B[b][1] <= 512 else 1
                        atps, sums = get_half(half)
                        emit_pv(h, b, tiles, atps, sums)
                        if b == halfA[-1]:
                            normalize(0)

                    prev = None
                    for b in range(NB):
                        tiles = emit_scores(h, b)
                        if interleave is not None:
                            interleave(b)
                        if prev is not None:
                            pv_for(*prev)
                        prev = (b, tiles)
                    pv_for(*prev)
                    normalize(1)

                    # AllGather doorbell on gpsimd (halves spilled above)
                    nc.gpsimd.collective_compute(
                        "AllGather",
                        mybir.AluOpType.bypass,
                        replica_groups=[list(range(N_CORES))],
                        ins=[agi.opt()],
                        outs=[ag_out[h][:]],
                    )

                # per-head: attention of head h front-loaded, with 4-chunk
                # slabs of head h+1's projection after each batch's scores;
                # the remaining chunks follow right after the attention
                SLAB = 4
                for h in range(QH_PER):
                    nxt = h + 1
                    pq = {}
                    pos = [0]          # chunks of head nxt emitted so far

                    def slab(b, nxt=nxt, pq=pq, pos=pos, n=SLAB):
                        if nxt >= QH_PER:
                            return
                        s0 = pos[0]
                        s1 = min(s0 + n, 2 * KCH)
                        for s in range(s0, s1):
                            half, k = divmod(s, KCH)
                            if half not in pq:
                                pq[half] = q_ps.tile(
                                    [128, 512], F32, tag="pq",
                                    name=f"pq_{nxt}_{half}",
                                )
                            proj_chunks(nxt, half, pq[half], k, k + 1)
                        pos[0] = s1

                    emit_attn(h, interleave=slab)
                    # rest of head nxt's projection; its ropes are emitted
                    # AFTER this head's normalize so the vector FIFO finishes
                    # attn h (and fires its AllGather) first
                    if nxt < QH_PER:
                        while pos[0] < 2 * KCH:
                            slab(None, n=2 * KCH - pos[0])
                        rope_q(nxt, 0, pq[0])
                        rope_q(nxt, 1, pq[1])
            esq.close()

            # ---- WO (column shard), consume gathers as they land ----------
            with ExitStack() as es3:
                ec3 = es3.enter_context
                af_pool = ec3(tc.tile_pool(name="af", bufs=3))
                osb_pool = ec3(tc.tile_pool(name="osb", bufs=2))
                wo_ps = ec3(tc.tile_pool(name="wops", bufs=1, space="PSUM"))
                pso = [
                    wo_ps.tile([128, T], F32, tag=f"o{ocb}",
                               name=f"wops_{ocb}")
                    for ocb in range(4)
                ]
                for i in range(H):
                    h, r = i // N_CORES, i % N_CORES
                    af = af_pool.tile([128, T], BF, tag="af",
                                      name=f"af_{i}")
                    nc.sync.dma_start(
                        af[:], ag_out[h][r * 128 : (r + 1) * 128, :]
                    )
                    for ocb in range(4):
                        for tt in range(2):
                            nc.tensor.matmul(
                                pso[ocb][:, tt * 512 : (tt + 1) * 512],
                                wosb[:, i, ocb * 128 : (ocb + 1) * 128],
                                af[:, tt * 512 : (tt + 1) * 512],
                                start=(i == 0),
                                stop=(i == H - 1),
                            )
                # evacuation split across scalar+vector (both idle here) so
                # the 8 half-copies drain in parallel instead of serially
                for ocb in range(4):
                    ob = osb_pool.tile([128, T], BF, tag="ob",
                                       name=f"ob_{ocb}")
                    for hh in range(2):
                        c0, c1 = hh * 512, (hh + 1) * 512
                        if (2 * ocb + hh) % 2 == 0:
                            nc.scalar.copy(ob[:, c0:c1], pso[ocb][:, c0:c1])
                        else:
                            nc.vector.tensor_copy(ob[:, c0:c1],
                                                  pso[ocb][:, c0:c1])
                        nc.sync.dma_start(
                            outT_d[ocb * 128 : (ocb + 1) * 128, c0:c1],
                            ob[:, c0:c1],
                        )

    nc.compile()
    return nc


def make_inputs(x, wqkv, wo, kv_cache, seqstarts, kvstarts, cachestarts,
                start_pos):
    """Host-side sharding: per-core input maps (bf16, partition-major)."""
    import ml_dtypes

    bf16 = ml_dtypes.bfloat16
    x = np.asarray(x, dtype=np.float32)
    wqkv = np.asarray(wqkv, dtype=np.float32)
    wo = np.asarray(wo, dtype=np.float32)
    kv_cache = np.asarray(kv_cache, dtype=np.float32)
    seqstarts = np.asarray(seqstarts)
    start_pos = np.asarray(start_pos)
    cachestarts = np.asarray(cachestarts)

    perm = np.concatenate([np.arange(0, HD, 2), np.arange(1, HD, 2)])
    # x: [T, HIDDEN] -> xT [HIDDEN, T] -> [128, KCH*T] p-major
    xT = np.ascontiguousarray(x.T)                       # [4096, 1024]
    xp = np.ascontiguousarray(
        xT.reshape(KCH, 128, T).transpose(1, 0, 2).reshape(128, KCH * T)
    ).astype(bf16)

    tok = np.arange(T)
    bq = np.clip(
        np.searchsorted(seqstarts, tok, side="right") - 1, 0,
        len(start_pos) - 1,
    )
    pos_q = tok - seqstarts[bq] + start_pos[bq]
    inv_freq = 1.0 / (ROPE_THETA ** (np.arange(D2, dtype=np.float64) / D2))
    ang = inv_freq[:, None] * pos_q[None, :].astype(np.float64)  # [64, T]
    cos = np.cos(ang).astype(np.float32)
    sin = np.sin(ang).astype(np.float32)
    s = np.float32(SCALE)
    top = np.concatenate([cos * s, sin * s, cos, sin], axis=1)  # [64, 4T]
    body = np.concatenate([top, np.zeros_like(top)], axis=0)    # [128, 4T]
    triT = np.where(
        np.arange(128)[:, None] > np.arange(128)[None, :], NEG, 0.0
    ).astype(np.float32)
    consts = np.concatenate([body, triT], axis=1).astype(bf16)

    sp_list = [int(v) for v in start_pos]
    CACHED = sum(sp_list)

    in_maps = []
    for c in range(N_CORES):
        # wqkv col-blocks [K(perm), V, Q0..Q3(perm)] packed [128, 6*KCH*128]
        cols = []
        kc = wqkv[:, HIDDEN + c * HD : HIDDEN + (c + 1) * HD]
        cols.append(kc[:, perm])
        cols.append(wqkv[:, HIDDEN + KVH * HD + c * HD
                         : HIDDEN + KVH * HD + (c + 1) * HD])
        for h in range(QH_PER):
            qh = 4 * c + h
            qc = wqkv[:, qh * HD : (qh + 1) * HD]
            cols.append(qc[:, perm])
        wq = np.stack(cols, 0)                       # [6, HIDDEN, 128]
        # [cb, (k p), c] -> [p, cb, k, c] -> [128, 6*KCH*128]
        wqkv_p = np.ascontiguousarray(
            wq.reshape(6, KCH, 128, 128).transpose(2, 0, 1, 3)
        ).reshape(128, 6 * KCH * 128).astype(bf16)

        # wo rows in kernel order i=(h,r): global head 4r+h, this core's cols
        wo_c = wo[:, 512 * c : 512 * (c + 1)]
        wo_p = np.empty((H, 128, 512), np.float32)
        for i in range(H):
            h, r = i // N_CORES, i % N_CORES
            g = 4 * r + h
            wo_p[i] = wo_c[g * 128 : (g + 1) * 128, :]
        # [i, p, c] -> [p, i*c]
        wo_p = np.ascontiguousarray(
            wo_p.transpose(1, 0, 2)
        ).reshape(128, H * 512).astype(bf16)

        # cached K/V: only the used slices, packed densely in batch order
        ckT_full = kv_cache[0, 0][:, c, :].T[perm]   # [128, 8192]
        cv_full = kv_cache[0, 1][:, c, :]            # [8192, 128]
        ck_parts, cv_parts = [], []
        for b, sp in enumerate(sp_list):
            if sp:
                cs = int(cachestarts[b])
                ck_parts.append(ckT_full[:, cs : cs + sp])
                # natural V rows -> [p, blk, c] p-major
                vb = cv_full[cs : cs + sp, :].reshape(sp // 128, 128, 128)
                cv_parts.append(
                    vb.transpose(1, 0, 2).reshape(128, sp)
                )
        if CACHED:
            ck_p = np.concatenate(ck_parts, axis=1)
            cv_p = np.concatenate(cv_parts, axis=1)
        else:
            ck_p = np.zeros((128, 0), np.float32)
            cv_p = np.zeros((128, 0), np.float32)
        if ck_p.shape[1] < 128:   # dram tensor min width
            pad = np.zeros((128, 128 - ck_p.shape[1]), np.float32)
            ck_p = np.concatenate([ck_p, pad], axis=1)
            cv_p = np.concatenate([cv_p, pad], axis=1)
        ck_p = np.ascontiguousarray(ck_p).astype(bf16)
        cv_p = np.ascontiguousarray(cv_p).astype(bf16)

        in_maps.append(dict(xp=xp, wqkv_p=wqkv_p, wo_p=wo_p, ck_p=ck_p,
                            cv_p=cv_p, consts=consts))
    return in_maps


_NC_CACHE = {}


def _get_nc(key, seqstarts, kvstarts, cachestarts, start_pos):
    if key not in _NC_CACHE:
        _NC_CACHE[key] = build_nc(seqstarts, kvstarts, cachestarts, start_pos)
    return _NC_CACHE[key]


def run(inputs, trace=False, tmpdir=None):
    """Build (cached), run on 8 cores, return (full_output, results)."""
    seqstarts = np.asarray(inputs["seqstarts"]).tolist()
    kvstarts = np.asarray(inputs["kvstarts"]).tolist()
    cachestarts = np.asarray(inputs["cachestarts"]).tolist()
    start_pos = np.asarray(inputs["start_pos"]).tolist()
    key = tuple(seqstarts) + tuple(kvstarts) + tuple(cachestarts) + tuple(
        start_pos
    )
    nc = _get_nc(key, seqstarts, kvstarts, cachestarts, start_pos)
    in_maps = make_inputs(
        inputs["x"], inputs["wqkv"], inputs["wo"], inputs["kv_cache"],
        seqstarts, kvstarts, cachestarts, start_pos,
    )
    kw = {}
    if trace:
        kw = dict(trace=True, tmpdir=tmpdir)
    res = run_bass_kernel_spmd(nc, in_maps, list(range(N_CORES)), **kw)
    out = np.empty((T, HIDDEN), dtype=np.float32)
    for c in range(N_CORES):
        out[:, 512 * c : 512 * (c + 1)] = (
            res.results[c]["outT"].astype(np.float32).T
        )
    return out, res


def kernel(**inputs) -> np.ndarray:
    out, _ = run(inputs)
    return out
